# revision 1
# baseline (speedup 1.0000x reference)
"""Trainium2 Bass kernel for nn_CTRModel (gnn_message_passing, KGAT-style CTR).

Strategy (8 NeuronCores, data-parallel over batch):
  - Each core handles 256 of the 2048 batch elements, for both the user and
    item sides and both (independent) attention layers: 4 "units" of
    256 batch x 32 neighbors = 8192 triples per unit.
  - node_emb is replicated; h-rows and t-rows are fetched with indirect
    (gather) DMAs of 1KB rows, 1024 rows per instruction.
  - Attention MLP: gathered h rows are PE-transposed to put the feature dim
    on partitions, then hid = relu(W1a.T @ hT + R2.T @ onehot(rel)) where
    R2[rel] = relation_emb[rel] @ W1b + b1 is folded host-side.
    att = W2 . hid via matmul; softmax over neighbors via exp (ACT) +
    block-diagonal-ones matmul (partition-group sums); the weighted t-sum is
    another block-diagonal matmul.
  - Large matmuls run as float32r (TF32-like) for 1 cycle/row.

Row bookkeeping per unit: row r = b*32 + k (b in [0,256), k in [0,32)).
Chunk cc covers r in [cc*1024, (cc+1)*1024); within a chunk, the SBUF
placement is r_local = s*128 + p (s in [0,8) "slot", p partition). For
b-local beta = b - 32*cc: s = beta//4, p = (beta%4)*32 + k.
"""
import os
import numpy as np

import concourse.bass as bass
import concourse.bacc as bacc
import concourse.mybir as mybir
from concourse.tile import TileContext

F32 = mybir.dt.float32
F32R = mybir.dt.float32r
I32 = mybir.dt.int32
AF = mybir.ActivationFunctionType

NCORES = 8
V = 100000          # nodes
NREL = 64
F = 4               # factors
D = 64
ROW = F * D         # 256 floats per node row
B = 2048
BC = B // NCORES    # 256 batch elems per core
K = 32              # neighbors
NL = 2              # layers
NUNITS = 4          # (side, layer)
CH = 8              # chunks per unit
G = 1024            # gathered rows per chunk
SLOTS = G // 128    # 8


def _r(ap):
    return ap.bitcast(F32R)


def build_nc():
    nc = bacc.Bacc("TRN2", target_bir_lowering=False, debug=False)

    node = nc.dram_tensor("node", [V, ROW], F32, kind="ExternalInput")
    hidx_d = nc.dram_tensor("hidx", [NUNITS, 128, CH * SLOTS], I32, kind="ExternalInput")
    tidx_d = nc.dram_tensor("tidx", [NUNITS, 128, CH * SLOTS], I32, kind="ExternalInput")
    rf_d = nc.dram_tensor("rf", [NUNITS, 16, 512], F32R, kind="ExternalInput")
    bidx_d = nc.dram_tensor("bidx", [128, 4], I32, kind="ExternalInput")
    wcat_d = nc.dram_tensor("wcat", [128, 128], F32R, kind="ExternalInput")
    r2c_d = nc.dram_tensor("r2c", [64, 256], F32R, kind="ExternalInput")
    w2c_d = nc.dram_tensor("w2c", [128, 2], F32R, kind="ExternalInput")
    bds_d = nc.dram_tensor("bds", [128, SLOTS, 32], F32R, kind="ExternalInput")
    bd4_d = nc.dram_tensor("bd4", [128, 4], F32, kind="ExternalInput")
    onest_d = nc.dram_tensor("onest", [4, 128], F32, kind="ExternalInput")
    ones1_d = nc.dram_tensor("ones1", [1, 64], F32R, kind="ExternalInput")
    iota_d = nc.dram_tensor("iota", [64, 1], F32, kind="ExternalInput")
    ident_d = nc.dram_tensor("ident", [128, 128], F32, kind="ExternalInput")

    out_d = nc.dram_tensor("out", [2, NL + 1, BC, ROW], F32, kind="ExternalOutput")

    with TileContext(nc) as tc:
        with (
            tc.tile_pool(name="const", bufs=1) as cpool,
            tc.tile_pool(name="hrow", bufs=2) as hpool,
            tc.tile_pool(name="trow", bufs=2) as tpool,
            tc.tile_pool(name="ht", bufs=3) as htpool,
            tc.tile_pool(name="hid", bufs=3) as hidpool,
            tc.tile_pool(name="oh", bufs=3) as ohpool,
            tc.tile_pool(name="vec", bufs=3) as vecpool,
            tc.tile_pool(name="sinv", bufs=2) as sinvpool,
            tc.tile_pool(name="osb", bufs=3) as opool,
            tc.tile_pool(name="psT", bufs=2, space="PSUM") as psT,
            tc.tile_pool(name="psH", bufs=2, space="PSUM") as psH,
            tc.tile_pool(name="psA", bufs=1, space="PSUM") as psA,
            tc.tile_pool(name="psS", bufs=2, space="PSUM") as psS,
            tc.tile_pool(name="psO", bufs=1, space="PSUM") as psO,
        ):
            # ---- constants into SBUF ----
            ident = cpool.tile([128, 128], F32)
            wcat = cpool.tile([128, 128], F32R)
            r2c = cpool.tile([64, 256], F32R)
            w2c = cpool.tile([128, 2], F32R)
            bds = cpool.tile([128, SLOTS, 32], F32R)
            bd4 = cpool.tile([128, 4], F32)
            onest = cpool.tile([4, 128], F32)
            ones1 = cpool.tile([1, 64], F32R)
            iota = cpool.tile([64, 1], F32)
            hidx = cpool.tile([128, NUNITS, CH * SLOTS], I32)
            tidx = cpool.tile([128, NUNITS, CH * SLOTS], I32)
            bidx = cpool.tile([128, 4], I32)

            for t, d in [(ident, ident_d), (wcat, wcat_d), (r2c, r2c_d),
                         (w2c, w2c_d), (bd4, bd4_d), (onest, onest_d),
                         (ones1, ones1_d), (iota, iota_d), (bidx, bidx_d)]:
                nc.sync.dma_start(out=t[:], in_=d[:])
            nc.sync.dma_start(out=bds[:], in_=bds_d[:])
            nc.sync.dma_start(out=hidx[:], in_=hidx_d[:].rearrange("u p c -> p u c"))
            nc.sync.dma_start(out=tidx[:], in_=tidx_d[:].rearrange("u p c -> p u c"))

            # ---- base embeddings (layer 0): plain gathers ----
            for side in range(0 if os.environ.get("KERN_NOBASE") else 2):
                bsb = opool.tile([128, 2, ROW], F32, tag="base")
                for j in range(2):
                    nc.gpsimd.indirect_dma_start(
                        out=bsb[:, j, :], out_offset=None, in_=node[:],
                        in_offset=bass.IndirectOffsetOnAxis(
                            ap=bidx[:, side * 2 + j:side * 2 + j + 1], axis=0),
                    )
                nc.sync.dma_start(
                    out=out_d[side, 0].rearrange("(p s) r -> p s r", s=2),
                    in_=bsb[:])

            # ---- attention units ----
            n_units = int(os.environ.get("KERN_UNITS", NUNITS))
            n_chunks = int(os.environ.get("KERN_CHUNKS", CH))
            stage = int(os.environ.get("KERN_STAGE", 99))
            for u in range(n_units):
                side, layer = divmod(u, NL)
                outsb = [opool.tile([128, ROW], F32, tag="osb", name=f"osb{u}_{h}")
                         for h in range(2)]
                for cc in range(n_chunks):
                    hrow = hpool.tile([128, SLOTS, ROW], F32)
                    trow = tpool.tile([128, SLOTS, ROW], F32)
                    for s in range(SLOTS):
                        nc.gpsimd.indirect_dma_start(
                            out=hrow[:, s, :], out_offset=None, in_=node[:],
                            in_offset=bass.IndirectOffsetOnAxis(
                                ap=hidx[:, u, cc * SLOTS + s:cc * SLOTS + s + 1],
                                axis=0),
                        )
                        nc.gpsimd.indirect_dma_start(
                            out=trow[:, s, :], out_offset=None, in_=node[:],
                            in_offset=bass.IndirectOffsetOnAxis(
                                ap=tidx[:, u, cc * SLOTS + s:cc * SLOTS + s + 1],
                                axis=0),
                        )

                    if stage < 2:
                        continue
                    # one-hot of relation ids: [64, 512] per half-chunk
                    ohs = []
                    for sg in range(2):
                        rfs = vecpool.tile([1, 512], F32R, tag="rf")
                        nc.sync.dma_start(out=rfs[:], in_=rf_d[u, cc * 2 + sg, :])
                        rbp = psH.tile([64, 512], F32, tag="psH")
                        nc.tensor.matmul(
                            out=rbp[:], lhsT=ones1[:], rhs=rfs[:],
                            start=True, stop=True, skip_group_check=True)
                        oh = ohpool.tile([64, 512], F32R)
                        nc.vector.tensor_tensor(
                            out=oh[:], in0=rbp[:],
                            in1=iota[:].to_broadcast([64, 512]),
                            op=mybir.AluOpType.is_equal)
                        ohs.append(oh)

                    if stage < 3:
                        continue
                    att = psA.tile([128, SLOTS, 4], F32)
                    hts = []
                    hids = []
                    for c in range(2):
                        ht = htpool.tile([128, SLOTS, 128], F32R)
                        hts.append(ht)
                        for sg in range(2):
                            tp = psT.tile([128, 512], F32)
                            for j in range(4):
                                s = sg * 4 + j
                                nc.tensor.transpose(
                                    out=tp[:, j * 128:(j + 1) * 128],
                                    in_=hrow[:, s, c * 128:(c + 1) * 128],
                                    identity=ident[:])
                            if c == 0:
                                nc.scalar.activation(
                                    out=ht[:, sg * 4:(sg + 1) * 4, :], in_=tp[:],
                                    func=AF.Copy)
                            else:
                                nc.vector.tensor_copy(
                                    out=ht[:, sg * 4:(sg + 1) * 4, :], in_=tp[:])

                        if stage < 4:
                            continue
                        hid = hidpool.tile([128, SLOTS, 128], F32R)
                        hids.append(hid)
                        for sg in range(2):
                            hp = psH.tile([128, 512], F32, tag="psH")
                            nc.tensor.matmul(
                                out=hp[:], lhsT=wcat[:],
                                rhs=ht[:, sg * 4:(sg + 1) * 4, :],
                                start=True, stop=False, skip_group_check=True)
                            nc.tensor.matmul(
                                out=hp[:], lhsT=r2c[:, c * 128:(c + 1) * 128],
                                rhs=ohs[sg][:],
                                start=False, stop=True, skip_group_check=True)
                            nc.scalar.activation(
                                out=hid[:, sg * 4:(sg + 1) * 4, :], in_=hp[:],
                                func=AF.Relu)
                        if stage < 5:
                            continue
                        for s in range(SLOTS):
                            nc.tensor.matmul(
                                out=att[:, s, 2 * c:2 * c + 2],
                                lhsT=hid[:, s, :], rhs=w2c[:],
                                start=True, stop=True, skip_group_check=True)

                    # softmax over k (k = partition mod 32 within 32-blocks)
                    if stage < 5:
                        continue
                    e_t = vecpool.tile([128, 32], F32, tag="E")
                    nc.scalar.activation(
                        out=e_t[:], in_=att[:].rearrange("p s f -> p (s f)"),
                        func=AF.Exp)
                    s_p = psS.tile([4, 32], F32, tag="psS")
                    nc.tensor.matmul(out=s_p[:], lhsT=bd4[:], rhs=e_t[:],
                                     start=True, stop=True, skip_group_check=True)
                    sinv = sinvpool.tile([4, 32], F32)
                    nc.vector.reciprocal(out=sinv[:], in_=s_p[:])
                    sb_p = psS.tile([128, 32], F32, tag="psS")
                    nc.tensor.matmul(out=sb_p[:], lhsT=onest[:], rhs=sinv[:],
                                     start=True, stop=True, skip_group_check=True)
                    w_t = vecpool.tile([128, 32], F32, tag="W")
                    nc.vector.tensor_tensor(out=w_t[:], in0=e_t[:], in1=sb_p[:],
                                            op=mybir.AluOpType.mult)

                    # weight t rows: trow[p, s, f, d] *= w[p, s, f]
                    if stage < 6:
                        continue
                    w_b = w_t[:].rearrange("p (s f o) -> p s f o", s=SLOTS, o=1) \
                        .to_broadcast([128, SLOTS, F, D])
                    wtb = tpool.tile([128, SLOTS, ROW], F32R, tag="wtb")
                    nc.vector.tensor_tensor(
                        out=wtb[:].rearrange("p s (f d) -> p s f d", f=F),
                        in0=trow[:].rearrange("p s (f d) -> p s f d", f=F),
                        in1=w_b, op=mybir.AluOpType.mult)

                    # sum over k: accumulate 8 block-diagonal matmuls
                    if stage < 7:
                        continue
                    tsp = psO.tile([32, ROW], F32)
                    for s in range(SLOTS):
                        nc.tensor.matmul(
                            out=tsp[:], lhsT=bds[:, s, :], rhs=wtb[:, s, :],
                            start=(s == 0), stop=(s == SLOTS - 1),
                            skip_group_check=True)
                    half, q = divmod(cc, 4)
                    nc.scalar.activation(
                        out=outsb[half][q * 32:(q + 1) * 32, :], in_=tsp[:],
                        func=AF.Copy)

                for half in range((n_chunks + 3) // 4):
                    if stage < 7:
                        nc.gpsimd.memset(outsb[half][:], 0.0)
                    nc.sync.dma_start(
                        out=out_d[side, 1 + layer, half * 128:(half + 1) * 128, :],
                        in_=outsb[half][:])

    nc.compile()
    return nc


def host_prep(users, items, users_h, users_r, users_t, items_h, items_r, items_t,
              node_emb, relation_emb, W1, b1, W2, b2):
    """Build per-core in_maps. Index tensors are re-laid-out host-side; small
    learned params are folded (R2 = relation_emb @ W1b + b1)."""
    node_flat = np.ascontiguousarray(np.asarray(node_emb, np.float32).reshape(V, ROW))
    W1 = np.asarray(W1, np.float32)
    b1 = np.asarray(b1, np.float32)
    W2 = np.asarray(W2, np.float32)
    W1a, W1b = W1[:D], W1[D:]

    wcat = np.zeros((128, 128), np.float32)
    wcat[:64, :64] = W1a
    wcat[64:, 64:] = W1a
    r2 = np.einsum("rfd,dj->rfj", np.asarray(relation_emb, np.float32), W1b) + b1
    r2c = np.ascontiguousarray(r2.reshape(64, 256))
    w2c = np.zeros((128, 2), np.float32)
    w2c[:64, 0] = W2[:, 0]
    w2c[64:, 1] = W2[:, 0]
    bds = np.zeros((128, SLOTS, 32), np.float32)
    for p in range(128):
        for s in range(SLOTS):
            bds[p, s, s * 4 + p // 32] = 1.0
    bd4 = np.zeros((128, 4), np.float32)
    bd4[np.arange(128), np.arange(128) // 32] = 1.0
    onest = np.zeros((4, 128), np.float32)
    onest[np.arange(128) // 32, np.arange(128)] = 1.0
    ones1 = np.ones((1, 64), np.float32)
    iota = np.arange(64, dtype=np.float32).reshape(64, 1)
    ident = np.eye(128, dtype=np.float32)

    def tile_idx(flat):  # [8192] -> [128, 64]
        return np.ascontiguousarray(
            flat.reshape(CH, SLOTS, 128).transpose(2, 0, 1).reshape(128, CH * SLOTS))

    h_all = [np.asarray(x, np.int32) for x in (users_h, items_h)]
    t_all = [np.asarray(x, np.int32) for x in (users_t, items_t)]
    r_all = [np.asarray(x, np.int32) for x in (users_r, items_r)]
    base = [np.asarray(users, np.int32), np.asarray(items, np.int32)]

    in_maps = []
    for c in range(NCORES):
        sl = slice(c * BC, (c + 1) * BC)
        hidx = np.zeros((NUNITS, 128, CH * SLOTS), np.int32)
        tidx = np.zeros((NUNITS, 128, CH * SLOTS), np.int32)
        rf = np.zeros((NUNITS, 16, 512), np.float32)
        for u in range(NUNITS):
            side, layer = divmod(u, NL)
            hidx[u] = tile_idx(h_all[side][layer, sl].reshape(-1))
            tidx[u] = tile_idx(t_all[side][layer, sl].reshape(-1))
            rf[u] = r_all[side][layer, sl].reshape(16, 512).astype(np.float32)
        bidx = np.stack([base[0][sl].reshape(128, 2), base[1][sl].reshape(128, 2)],
                        axis=1).reshape(128, 4)
        in_maps.append({
            "node": node_flat, "hidx": hidx, "tidx": tidx, "rf": rf,
            "bidx": np.ascontiguousarray(bidx),
            "wcat": wcat, "r2c": r2c, "w2c": w2c, "bds": bds, "bd4": bd4,
            "onest": onest, "ones1": ones1, "iota": iota, "ident": ident,
        })
    return in_maps


_NC_CACHE = None
LAST_RESULT = None


def kernel(**inputs):
    global _NC_CACHE, LAST_RESULT
    from concourse.bass_utils import run_bass_kernel_spmd

    if _NC_CACHE is None:
        _NC_CACHE = build_nc()
    nc = _NC_CACHE

    in_maps = host_prep(**inputs)
    res = run_bass_kernel_spmd(nc, in_maps, core_ids=list(range(NCORES)))
    LAST_RESULT = res

    user = np.concatenate([r["out"][0] for r in res.results], axis=1)
    item = np.concatenate([r["out"][1] for r in res.results], axis=1)
    user = user.reshape(NL + 1, B, F, D)
    item = item.reshape(NL + 1, B, F, D)
    return user, item

